# revision 1
# baseline (speedup 1.0000x reference)
"""Trainium2 Bass kernel for nn_ContrastiveLoss (ragged_sequence).

Math (see reference): a cross-attention t2i score matrix scores[i, c] over
B=64 images x B=64 captions, then a max-violation margin loss.

Sharding: captions are sharded 8-per-core across 8 NeuronCores; images are
replicated.  Each core computes its [64, 8] column block of the score
matrix; the tiny 64x64 margin-loss reduction runs on host.

Device layout (per core):
  Images are processed in 22 "packs" of 3 images (64 real + 2 zero-pad).
  A pack occupies 108 SBUF partitions = 3 images x 36 regions.  The main
  matmul A = im @ s^T is computed with stationary = im-pack [128d, 108br]
  (fp32r, 8 K-chunks of d) and moving = s^T [128d, 400cw], giving
  A [108 (b,r), 400 (c,w)] in PSUM.  All word-axis (w) reductions are
  free-axis DVE reduces; all region-axis (r) reductions are matmuls with
  block-diagonal stationaries:
    - H = Gbd @ E with Gbd = blockdiag(G[b0],G[b1],G[b2]) (Gram matrices)
    - Z/numZ/wsqZ2 = ones_p^T @ {E, E*A, E*H} where ones_p [108, 66] has its
      3 block-ones columns at rows 3p, accumulating all 22 packs directly
      into persistent [66, 400] PSUM tiles (no per-pack evacuation).
  sqrt is computed as exp(0.5*ln(x)) so every activation (Prelu-free path
  uses DVE leaky) stays inside one ACT table set (natural_log_exp).
"""

import sys

if "/opt/trn_rl_repo" not in sys.path:
    sys.path.insert(0, "/opt/trn_rl_repo")

import numpy as np

B, R, W, D = 64, 36, 50, 1024
NCORES = 8
CPC = B // NCORES          # captions per core = 8
NCW = CPC * W              # 400 = per-core (c, w) columns
PACK = 3                   # images per pack
NPACK = 22                 # ceil(64 / 3) -> 66 rows incl. 2 pad images
BP = NPACK * PACK          # 66
PPART = PACK * R           # 108 partitions per pack
KCH = D // 128             # 8 contraction chunks

MARGIN = 0.2
LAM_SM = 9.0
LAM_LSE = 6.0
EPS = 1e-8

_PROGRAM_CACHE: dict = {}


def build_program(debug: bool = False, leaky_on_act: bool = True):
    """Build the per-core Bass/Tile program (identical on all 8 cores)."""
    import concourse.bacc as bacc
    import concourse.mybir as mybir
    import concourse.tile as tile

    f32 = mybir.dt.float32
    f32r = mybir.dt.float32r
    AF = mybir.ActivationFunctionType
    ALU = mybir.AluOpType
    AX = mybir.AxisListType

    nc = bacc.Bacc("TRN2", target_bir_lowering=False, debug=debug)

    im_d = nc.dram_tensor("im_packed", [NPACK, 128, KCH * PPART], f32, kind="ExternalInput")
    s_d = nc.dram_tensor("s_packed", [128, KCH * NCW], f32, kind="ExternalInput")
    g_d = nc.dram_tensor("gbd", [NPACK, PPART, PPART], f32, kind="ExternalInput")
    o_d = nc.dram_tensor("ones_p", [NPACK, PPART, BP], f32, kind="ExternalInput")
    cn_d = nc.dram_tensor("cn66", [BP, NCW], f32, kind="ExternalInput")
    pc_d = nc.dram_tensor("padc66", [BP, CPC], f32, kind="ExternalInput")
    out_d = nc.dram_tensor("scores8", [B, CPC], f32, kind="ExternalOutput")

    with tile.TileContext(nc) as tc:
        with (
            tc.tile_pool(name="const", bufs=1) as cpool,
            tc.tile_pool(name="imp", bufs=4) as impool,
            tc.tile_pool(name="gop", bufs=4) as gopool,
            tc.tile_pool(name="work", bufs=3) as work,
            tc.tile_pool(name="small", bufs=3) as small,
            tc.tile_pool(name="ph2", bufs=1) as ph2,
            tc.tile_pool(name="psA", bufs=2, space="PSUM") as psA,
            tc.tile_pool(name="psH", bufs=2, space="PSUM") as psH,
            tc.tile_pool(name="psacc", bufs=1, space="PSUM") as psacc,
        ):
            s_sb = cpool.tile([128, KCH * NCW], f32)
            nc.sync.dma_start(s_sb[:].bitcast(f32r), s_d[:].bitcast(f32r))
            cn_sb = cpool.tile([BP, NCW], f32)
            nc.sync.dma_start(cn_sb[:], cn_d[:])
            pc_sb = cpool.tile([BP, CPC], f32)
            nc.sync.dma_start(pc_sb[:], pc_d[:])

            # persistent PSUM accumulators for the r-reductions
            z_acc = psacc.tile([BP, NCW], f32)
            nz_acc = psacc.tile([BP, NCW], f32)
            wz_acc = psacc.tile([BP, NCW], f32)

            for p in range(NPACK):
                im_sb = impool.tile([128, KCH * PPART], f32, tag="im")
                nc.sync.dma_start(im_sb[:].bitcast(f32r), im_d[p].bitcast(f32r))
                g_sb = gopool.tile([PPART, PPART], f32, tag="g")
                nc.sync.dma_start(g_sb[:].bitcast(f32r), g_d[p].bitcast(f32r))
                o_sb = gopool.tile([PPART, BP], f32, tag="o")
                nc.sync.dma_start(o_sb[:].bitcast(f32r), o_d[p].bitcast(f32r))

                first, last = (p == 0), (p == NPACK - 1)

                # A[108, 400] = sum_k im_pack_k^T @ s_k  (fp32r)
                a_ps = psA.tile([PPART, NCW], f32)
                for k in range(KCH):
                    nc.tensor.matmul(
                        a_ps[:],
                        im_sb[:, k * PPART:(k + 1) * PPART].bitcast(f32r),
                        s_sb[:, k * NCW:(k + 1) * NCW].bitcast(f32r),
                        start=(k == 0),
                        stop=(k == KCH - 1),
                    )

                # Al = leaky_relu(A, 0.1) on ACT (Prelu, same table set).
                # CoreSim lacks Prelu, so tests can fall back to a DVE path.
                al = work.tile([PPART, NCW], f32, tag="al")
                if leaky_on_act:
                    nc.scalar.activation(al[:], a_ps[:], AF.Prelu, alpha=0.1)
                else:
                    a_sb = work.tile([PPART, NCW], f32, tag="asb")
                    nc.scalar.activation(a_sb[:], a_ps[:], AF.Copy)
                    nc.vector.scalar_tensor_tensor(
                        al[:], a_sb[:], 0.1, a_sb[:], ALU.mult, ALU.max
                    )

                # s2[108, 8] = sum_w Al^2
                sq = work.tile([PPART, NCW], f32, tag="sq")
                nc.scalar.activation(sq[:], al[:], AF.Square)
                s2 = small.tile([PPART, CPC], f32, tag="s2")
                nc.vector.tensor_reduce(
                    s2[:], sq[:].rearrange("p (c w) -> p c w", c=CPC), AX.X, ALU.add
                )

                # rnrm = rsqrt(max(s2,1e-30)) = exp(-0.5*ln(.)); matches the
                # reference 1/(sqrt(s2)+1e-8) to ~1e-10 rel on valid columns
                s2m = small.tile([PPART, CPC], f32, tag="s2m")
                nc.vector.tensor_scalar_max(s2m[:], s2[:], 1e-30)
                lns = small.tile([PPART, CPC], f32, tag="lns")
                nc.scalar.activation(lns[:], s2m[:], AF.Ln)
                rnrm = small.tile([PPART, CPC], f32, tag="rnrm")
                nc.scalar.activation(rnrm[:], lns[:], AF.Exp, scale=-0.5)

                # An = Al * rnrm (broadcast over w); E = exp(9*An)
                an = work.tile([PPART, NCW], f32, tag="an")
                nc.vector.tensor_mul(
                    an[:].rearrange("p (c w) -> p c w", c=CPC),
                    al[:].rearrange("p (c w) -> p c w", c=CPC),
                    rnrm[:].broadcast_to([PPART, CPC, W]),
                )
                e = work.tile([PPART, NCW], f32, tag="e")
                nc.scalar.activation(e[:].bitcast(f32r), an[:], AF.Exp, scale=LAM_SM)

                # H = Gbd @ E ; Z += ones_p^T E
                h_ps = psH.tile([PPART, NCW], f32)
                nc.tensor.matmul(
                    h_ps[:], g_sb[:].bitcast(f32r), e[:].bitcast(f32r),
                    start=True, stop=True,
                )
                nc.tensor.matmul(
                    z_acc[:], o_sb[:].bitcast(f32r), e[:].bitcast(f32r),
                    start=first, stop=last,
                )

                # EA = E * A ; numZ += ones_p^T EA
                ea = work.tile([PPART, NCW], f32, tag="ea")
                nc.vector.tensor_mul(ea[:].bitcast(f32r), e[:], a_ps[:])
                nc.tensor.matmul(
                    nz_acc[:], o_sb[:].bitcast(f32r), ea[:].bitcast(f32r),
                    start=first, stop=last,
                )

                # EH = E * H ; wsqZ2 += ones_p^T EH
                eh = work.tile([PPART, NCW], f32, tag="eh")
                nc.vector.tensor_mul(eh[:].bitcast(f32r), e[:], h_ps[:])
                nc.tensor.matmul(
                    wz_acc[:], o_sb[:].bitcast(f32r), eh[:].bitcast(f32r),
                    start=first, stop=last,
                )

            # ---- phase 2: per-(b, c, w) epilogue on [66, 400] tiles ----
            zs = ph2.tile([BP, NCW], f32)
            nc.scalar.activation(zs[:], z_acc[:], AF.Copy)
            ns = ph2.tile([BP, NCW], f32)
            nc.scalar.activation(ns[:], nz_acc[:], AF.Copy)
            ws = ph2.tile([BP, NCW], f32)
            nc.scalar.activation(ws[:], wz_acc[:], AF.Copy)

            rz = ph2.tile([BP, NCW], f32)
            nc.vector.reciprocal(rz[:], zs[:])
            num = ph2.tile([BP, NCW], f32)
            nc.vector.tensor_mul(num[:], ns[:], rz[:])
            wt = ph2.tile([BP, NCW], f32)
            nc.vector.tensor_mul(wt[:], ws[:], rz[:])
            wsq = ph2.tile([BP, NCW], f32)
            nc.vector.tensor_mul(wsq[:], wt[:], rz[:])

            # wn = sqrt(max(wsq, 0)) via exp(0.5*ln(max(wsq, 1e-30)))
            wsqm = ph2.tile([BP, NCW], f32)
            nc.vector.tensor_scalar_max(wsqm[:], wsq[:], 1e-30)
            lnw = ph2.tile([BP, NCW], f32)
            nc.scalar.activation(lnw[:], wsqm[:], AF.Ln)
            wn = ph2.tile([BP, NCW], f32)
            nc.scalar.activation(wn[:], lnw[:], AF.Exp, scale=0.5)

            # sim = num / max(cn * wn, eps) ; ee = exp(6*sim)
            den = ph2.tile([BP, NCW], f32)
            nc.vector.tensor_mul(den[:], cn_sb[:], wn[:])
            den2 = ph2.tile([BP, NCW], f32)
            nc.vector.tensor_scalar_max(den2[:], den[:], EPS)
            rden = ph2.tile([BP, NCW], f32)
            nc.vector.reciprocal(rden[:], den2[:])
            simt = ph2.tile([BP, NCW], f32)
            nc.vector.tensor_mul(simt[:], num[:], rden[:])
            ee = ph2.tile([BP, NCW], f32)
            nc.scalar.activation(ee[:], simt[:], AF.Exp, scale=LAM_LSE)

            # rowZ = sum_w ee ; padded words contribute exactly 1 each ->
            # subtract the per-caption pad count, then log()/6.
            rowz = ph2.tile([BP, CPC], f32)
            nc.vector.tensor_reduce(
                rowz[:], ee[:].rearrange("p (c w) -> p c w", c=CPC), AX.X, ALU.add
            )
            rowc = ph2.tile([BP, CPC], f32)
            nc.vector.tensor_sub(rowc[:], rowz[:], pc_sb[:])
            lnr = ph2.tile([BP, CPC], f32)
            nc.scalar.activation(lnr[:], rowc[:], AF.Ln)
            sc = ph2.tile([BP, CPC], f32)
            nc.scalar.mul(sc[:], lnr[:], 1.0 / LAM_LSE)

            nc.sync.dma_start(out_d[:], sc[0:B, :])

    nc.compile()
    return nc


def prepare_inputs(im: np.ndarray, s: np.ndarray, s_l: np.ndarray):
    """Host-side input marshalling: shard captions, transpose to d-major,
    pack images into 3-image/108-partition packs, build the block-diagonal
    Gram and ones stationaries, caption norms and pad counts."""
    im = np.ascontiguousarray(np.asarray(im, np.float32))
    s = np.ascontiguousarray(np.asarray(s, np.float32))
    s_l = np.asarray(s_l).astype(np.int64)

    # zero out padded words so A columns for padded (c, w) are exactly 0
    wmask = (np.arange(W)[None, :] < s_l[:, None])          # [64, 50]
    s_z = s * wmask[:, :, None].astype(np.float32)

    # im packs: [22, 128, 8*108]
    imf = im.transpose(2, 0, 1).reshape(D, B * R)            # [1024, 2304]
    imf66 = np.zeros((D, BP * R), np.float32)
    imf66[:, : B * R] = imf
    im_packed = np.ascontiguousarray(
        imf66.reshape(KCH, 128, NPACK, PPART)
        .transpose(2, 1, 0, 3)
        .reshape(NPACK, 128, KCH * PPART)
    )

    # Gram matrices, block-diagonal per pack: [22, 108, 108]
    G = np.matmul(im, im.transpose(0, 2, 1))                 # [64, 36, 36] f32
    gbd = np.zeros((NPACK, PPART, PPART), np.float32)
    for j in range(PACK):
        for p in range(NPACK):
            b = PACK * p + j
            if b < B:
                gbd[p, R * j : R * (j + 1), R * j : R * (j + 1)] = G[b]

    # ones_p stationaries: [22, 108, 66], 3 block-ones columns at 3p
    ones_p = np.zeros((NPACK, PPART, BP), np.float32)
    for p in range(NPACK):
        for j in range(PACK):
            ones_p[p, R * j : R * (j + 1), PACK * p + j] = 1.0

    # caption norms (from zeroed s -> 0 at padded words) and pad counts
    cn = np.sqrt((s_z * s_z).sum(axis=2))                    # [64, 50]
    padc = (W - s_l).astype(np.float32)                      # [64]

    in_maps = []
    for c in range(NCORES):
        cs = slice(CPC * c, CPC * (c + 1))
        s_cc = s_z[cs]                                        # [8, 50, 1024]
        sT = s_cc.transpose(2, 0, 1).reshape(D, NCW)          # [1024, 400]
        s_packed = np.ascontiguousarray(
            sT.reshape(KCH, 128, NCW).transpose(1, 0, 2).reshape(128, KCH * NCW)
        )
        cn66 = np.broadcast_to(cn[cs].reshape(1, NCW), (BP, NCW))
        padc66 = np.broadcast_to(padc[cs].reshape(1, CPC), (BP, CPC))
        in_maps.append(
            {
                "im_packed": im_packed,
                "s_packed": s_packed,
                "gbd": gbd,
                "ones_p": ones_p,
                "cn66": np.ascontiguousarray(cn66, dtype=np.float32),
                "padc66": np.ascontiguousarray(padc66, dtype=np.float32),
            }
        )
    return in_maps


def margin_loss(scores: np.ndarray) -> np.float32:
    scores = scores.astype(np.float32)
    diag = np.diag(scores).copy()
    cost_s = np.maximum(MARGIN + scores - diag[:, None], 0.0)
    cost_im = np.maximum(MARGIN + scores - diag[None, :], 0.0)
    np.fill_diagonal(cost_s, 0.0)
    np.fill_diagonal(cost_im, 0.0)
    return np.float32(cost_s.max(axis=1).sum() + cost_im.max(axis=0).sum())


def kernel(im: np.ndarray, s: np.ndarray, s_l: np.ndarray) -> np.ndarray:
    from concourse.bass_utils import run_bass_kernel_spmd

    if "nc" not in _PROGRAM_CACHE:
        _PROGRAM_CACHE["nc"] = build_program()
    nc = _PROGRAM_CACHE["nc"]

    in_maps = prepare_inputs(im, s, s_l)
    res = run_bass_kernel_spmd(nc, in_maps, list(range(NCORES))).results
    scores = np.concatenate([res[c]["scores8"] for c in range(NCORES)], axis=1)
    return margin_loss(scores)



# revision 2
# speedup vs baseline: 1.2503x; 1.2503x over previous
"""Trainium2 Bass kernel for nn_ContrastiveLoss (ragged_sequence), v2.

Math (see reference): a cross-attention t2i score matrix scores[i, c] over
B=64 images x B=64 captions, then a max-violation margin loss.

Sharding: captions are sharded 8-per-core across 8 NeuronCores; images are
replicated.  Each core computes its [64, 8] column block of the score
matrix; the tiny 64x64 margin-loss reduction runs on host.

v2 changes vs v1 (162 us -> target ~50 us):
  * all matmul operands in bf16 (v1's f32r bitcast silently lowered to
    fp32_mode=HIGH multi-pass matmuls at ~3x the cycles)
  * the only ACT functions in the main loop are Prelu / Copy / Exp, which
    all live in the exp_and_others table set -> no ACT_TABLE_LOAD thrash
    (v1 spent 59 us swapping ln/exp tables every pack)
  * rsqrt for the word-axis normalization is a Newton iteration on DVE
    (magic-constant seed + 2 steps), batched over groups of 4 packs
  * Z-cancellation: softmax denominator Z cancels in sim =
    (num/Z)/(cn*sqrt(ws)/Z) = ns/(cn*sqrt(ws)); the Z accumulator matmul,
    phase-2 reciprocals and several elementwise ops are gone
  * elementwise work split across ACT (leaky, A evac, exp), DVE (squares,
    reduce, newton, e*A, e*H) and GPSIMD (an = al * rnrm)
  * one fused im|G|ones DMA per pack; final ln()/6 moved to host
"""

import sys

if "/opt/trn_rl_repo" not in sys.path:
    sys.path.insert(0, "/opt/trn_rl_repo")

import numpy as np

B, R, W, D = 64, 36, 50, 1024
NCORES = 8
CPC = B // NCORES          # captions per core = 8
NCW = CPC * W              # 400 = per-core (c, w) columns
PACK = 3                   # images per pack
NPACK = 22                 # ceil(64 / 3) -> 66 rows incl. 2 pad images
BP = NPACK * PACK          # 66
PPART = PACK * R           # 108 partitions per pack
KCH = D // 128             # 8 contraction chunks
GO = PPART + BP            # 174 = G|ones combined columns
IMC = KCH * PPART          # 864 im columns per pack
PKC = IMC + GO             # 1038 combined pack columns

GSIZE = 4                  # packs per newton batch

MARGIN = 0.2
LAM_SM = 9.0
LAM_LSE = 6.0

AN_ON_GPSIMD = True        # an = al * rnrm9 on GPSIMD (else DVE)

_PROGRAM_CACHE: dict = {}


def build_program(debug: bool = False):
    """Build the per-core Bass/Tile program (identical on all 8 cores)."""
    import concourse.bacc as bacc
    import concourse.mybir as mybir
    import concourse.tile as tile

    f32 = mybir.dt.float32
    bf16 = mybir.dt.bfloat16
    i32 = mybir.dt.int32
    AF = mybir.ActivationFunctionType
    ALU = mybir.AluOpType
    AX = mybir.AxisListType

    nc = bacc.Bacc("TRN2", target_bir_lowering=False, debug=debug)

    pk_d = nc.dram_tensor("pk", [NPACK, 128, PKC], bf16, kind="ExternalInput")
    s_d = nc.dram_tensor("s_packed", [128, KCH * NCW], bf16, kind="ExternalInput")
    nlc_d = nc.dram_tensor("neglncn", [BP, NCW], f32, kind="ExternalInput")
    pc_d = nc.dram_tensor("padc66", [BP, CPC], f32, kind="ExternalInput")
    out_d = nc.dram_tensor("scores8", [B, CPC], f32, kind="ExternalOutput")

    groups = [list(range(g, min(g + GSIZE, NPACK))) for g in range(0, NPACK, GSIZE)]

    with tile.TileContext(nc) as tc:
        with (
            tc.tile_pool(name="const", bufs=1) as cpool,
            tc.tile_pool(name="pk", bufs=4) as pkpool,
            tc.tile_pool(name="ala", bufs=2 * GSIZE + 2) as alpool,
            tc.tile_pool(name="sqp", bufs=3) as sqpool,
            tc.tile_pool(name="anp", bufs=3) as anpool,
            tc.tile_pool(name="ep", bufs=4) as epool,
            tc.tile_pool(name="eaeh", bufs=3) as eapool,
            tc.tile_pool(name="nwt", bufs=2) as nwt,
            tc.tile_pool(name="ph2", bufs=1) as ph2,
            tc.tile_pool(name="psA", bufs=3, space="PSUM") as psA,
            tc.tile_pool(name="psH", bufs=2, space="PSUM") as psH,
            tc.tile_pool(name="psacc", bufs=1, space="PSUM") as psacc,
        ):
            s_sb = cpool.tile([128, KCH * NCW], bf16)
            nc.sync.dma_start(s_sb[:], s_d[:])
            nlc_sb = cpool.tile([BP, NCW], f32)
            nc.sync.dma_start(nlc_sb[:], nlc_d[:])
            pc_sb = cpool.tile([BP, CPC], f32)
            nc.sync.dma_start(pc_sb[:], pc_d[:])

            # magic constant for the rsqrt newton seed
            magic = cpool.tile([PPART, 1], i32)
            nc.vector.memset(magic[:], 0x5F3759DF)

            # persistent per-(b,r) x caption tiles for s2 / 9*rsqrt(s2)
            s2_all = cpool.tile([PPART, NPACK * CPC], f32)
            rn9_all = cpool.tile([PPART, NPACK * CPC], f32)

            # persistent PSUM accumulators (fp32): ns = sum_r E*A,
            # ws = sum_r E*H, accumulated over all 22 packs
            nz_acc = psacc.tile([BP, NCW], f32)
            wz_acc = psacc.tile([BP, NCW], f32)

            al_t: dict = {}
            asb_t: dict = {}
            pk_t: dict = {}

            for gi, packs in enumerate(groups):
                # ---- sweep 1: A matmuls, leaky, squares, word-norms ----
                for p in packs:
                    pk_sb = pkpool.tile([128, PKC], bf16, tag="pk")
                    nc.sync.dma_start(pk_sb[:], pk_d[p])
                    pk_t[p] = pk_sb

                    # A[108, 400] = sum_k im_pack_k^T @ s_k  (bf16)
                    a_ps = psA.tile([PPART, NCW], f32)
                    for k in range(KCH):
                        nc.tensor.matmul(
                            a_ps[:],
                            pk_sb[:, k * PPART:(k + 1) * PPART],
                            s_sb[:, k * NCW:(k + 1) * NCW],
                            start=(k == 0),
                            stop=(k == KCH - 1),
                        )

                    # al = leaky_relu(A, 0.1) and a copy of raw A, both ->
                    # SBUF bf16 (ACT; Prelu/Copy live in the exp table set)
                    al = alpool.tile([PPART, NCW], bf16, tag="al")
                    nc.scalar.activation(al[:], a_ps[:], AF.Prelu, alpha=0.1)
                    a_sb = alpool.tile([PPART, NCW], bf16, tag="asb")
                    nc.scalar.activation(a_sb[:], a_ps[:], AF.Copy)
                    al_t[p], asb_t[p] = al, a_sb

                    # s2[108, 8] = sum_w al^2
                    sq = sqpool.tile([PPART, NCW], bf16, tag="sq")
                    nc.vector.tensor_mul(sq[:], al[:], al[:])
                    nc.vector.tensor_reduce(
                        s2_all[:, p * CPC:(p + 1) * CPC],
                        sq[:].rearrange("p (c w) -> p c w", c=CPC),
                        AX.X,
                        ALU.add,
                    )

                # ---- newton rsqrt batch over this group's captions ----
                lo, hi = packs[0] * CPC, (packs[-1] + 1) * CPC
                n = hi - lo
                x = nwt.tile([PPART, n], f32, tag="x")
                nc.vector.tensor_scalar_max(x[:], s2_all[:, lo:hi], 1e-30)
                t1 = nwt.tile([PPART, n], i32, tag="t1")
                nc.vector.tensor_scalar(
                    t1[:], x[:].bitcast(i32), 1, None, op0=ALU.logical_shift_right
                )
                y0 = nwt.tile([PPART, n], f32, tag="y0")
                nc.vector.tensor_tensor(
                    y0[:].bitcast(i32),
                    magic[:].broadcast_to([PPART, n]),
                    t1[:],
                    op=ALU.subtract,
                )
                # step 1: y1 = y0 * (1.5 - 0.5 * x * y0^2)
                a1 = nwt.tile([PPART, n], f32, tag="a1")
                nc.vector.tensor_mul(a1[:], y0[:], y0[:])
                nc.vector.tensor_mul(a1[:], a1[:], x[:])
                nc.vector.tensor_scalar(a1[:], a1[:], -0.5, 1.5, op0=ALU.mult, op1=ALU.add)
                y1 = nwt.tile([PPART, n], f32, tag="y1")
                nc.vector.tensor_mul(y1[:], a1[:], y0[:])
                # step 2 (folding in the 9x softmax temperature):
                # rn9 = y1 * (13.5 - 4.5 * x * y1^2) = 9 * rsqrt(x)
                b1 = nwt.tile([PPART, n], f32, tag="b1")
                nc.vector.tensor_mul(b1[:], y1[:], y1[:])
                nc.vector.tensor_mul(b1[:], b1[:], x[:])
                nc.vector.tensor_scalar(b1[:], b1[:], -4.5, 13.5, op0=ALU.mult, op1=ALU.add)
                nc.vector.tensor_mul(rn9_all[:, lo:hi], b1[:], y1[:])

                # ---- sweep 2: E, H, E*A, E*H, the two r-reductions ----
                for p in packs:
                    first, last = (p == 0), (p == NPACK - 1)
                    al, a_sb, pk_sb = al_t.pop(p), asb_t.pop(p), pk_t.pop(p)

                    an = anpool.tile([PPART, NCW], f32, tag="an")
                    eng = nc.gpsimd if AN_ON_GPSIMD else nc.vector
                    eng.tensor_mul(
                        an[:].rearrange("p (c w) -> p c w", c=CPC),
                        al[:].rearrange("p (c w) -> p c w", c=CPC),
                        rn9_all[:, p * CPC:(p + 1) * CPC].broadcast_to(
                            [PPART, CPC, W]
                        ),
                    )
                    e = epool.tile([PPART, NCW], bf16, tag="e")
                    nc.scalar.activation(e[:], an[:], AF.Exp)

                    # H = Gbd @ E
                    h_ps = psH.tile([PPART, NCW], f32)
                    nc.tensor.matmul(
                        h_ps[:], pk_sb[:PPART, IMC:IMC + PPART], e[:],
                        start=True, stop=True,
                    )

                    # ea = E*A ; ns += ones^T ea
                    ea = eapool.tile([PPART, NCW], bf16, tag="ea")
                    nc.vector.tensor_mul(ea[:], e[:], a_sb[:])
                    nc.tensor.matmul(
                        nz_acc[:], pk_sb[:PPART, IMC + PPART:IMC + GO], ea[:],
                        start=first, stop=last,
                    )

                    # eh = E*H ; ws += ones^T eh
                    eh = eapool.tile([PPART, NCW], bf16, tag="eh")
                    nc.vector.tensor_mul(eh[:], e[:], h_ps[:])
                    nc.tensor.matmul(
                        wz_acc[:], pk_sb[:PPART, IMC + PPART:IMC + GO], eh[:],
                        start=first, stop=last,
                    )

            # ---- phase 2: sim = ns / (cn * sqrt(ws)), LSE over words ----
            wsm = ph2.tile([BP, NCW], f32)
            nc.vector.tensor_scalar_max(wsm[:], wz_acc[:], 1e-30)
            t = ph2.tile([BP, NCW], f32)
            nc.scalar.activation(t[:], wsm[:], AF.Ln)
            # u = -0.5*ln(ws) - ln(cn)
            u = ph2.tile([BP, NCW], f32)
            nc.vector.scalar_tensor_tensor(
                u[:], t[:], -0.5, nlc_sb[:], ALU.mult, ALU.add
            )
            q = ph2.tile([BP, NCW], f32)
            nc.scalar.activation(q[:], u[:], AF.Exp)
            sim = ph2.tile([BP, NCW], f32)
            nc.vector.tensor_mul(sim[:], q[:], nz_acc[:])
            ee = ph2.tile([BP, NCW], f32)
            nc.scalar.activation(ee[:], sim[:], AF.Exp, scale=LAM_LSE)

            # rowz = sum_w ee ; padded words contribute exactly 1 each
            rowz = ph2.tile([BP, CPC], f32)
            nc.vector.tensor_reduce(
                rowz[:], ee[:].rearrange("p (c w) -> p c w", c=CPC), AX.X, ALU.add
            )
            rowc = ph2.tile([BP, CPC], f32)
            nc.vector.tensor_sub(rowc[:], rowz[:], pc_sb[:])

            # host finishes with ln(rowc)/6
            nc.sync.dma_start(out_d[:], rowc[0:B, :])

    nc.compile()
    return nc


def prepare_inputs(im: np.ndarray, s: np.ndarray, s_l: np.ndarray):
    """Host-side input marshalling: shard captions, transpose to d-major,
    pack images into 3-image/108-partition packs, build the block-diagonal
    Gram + ones stationaries (fused with im into one per-pack DMA buffer),
    -ln(caption norms) and pad counts."""
    import ml_dtypes

    bf16 = ml_dtypes.bfloat16
    im = np.ascontiguousarray(np.asarray(im, np.float32))
    s = np.ascontiguousarray(np.asarray(s, np.float32))
    s_l = np.asarray(s_l).astype(np.int64)

    # zero out padded words so A columns for padded (c, w) are exactly 0
    wmask = (np.arange(W)[None, :] < s_l[:, None])          # [64, 50]
    s_z = s * wmask[:, :, None].astype(np.float32)

    # im packs: [22, 128, 864]
    imf = im.transpose(2, 0, 1).reshape(D, B * R)            # [1024, 2304]
    imf66 = np.zeros((D, BP * R), np.float32)
    imf66[:, : B * R] = imf
    im_packed = (
        imf66.reshape(KCH, 128, NPACK, PPART)
        .transpose(2, 1, 0, 3)
        .reshape(NPACK, 128, IMC)
    )

    # Gram matrices, block-diagonal per pack: [22, 108, 108]
    G = np.matmul(im, im.transpose(0, 2, 1))                 # [64, 36, 36] f32
    gbd = np.zeros((NPACK, PPART, PPART), np.float32)
    for j in range(PACK):
        for p in range(NPACK):
            b = PACK * p + j
            if b < B:
                gbd[p, R * j: R * (j + 1), R * j: R * (j + 1)] = G[b]

    # ones_p stationaries: [22, 108, 66], 3 block-ones columns at 3p
    ones_p = np.zeros((NPACK, PPART, BP), np.float32)
    for p in range(NPACK):
        for j in range(PACK):
            ones_p[p, R * j: R * (j + 1), PACK * p + j] = 1.0

    # fused per-pack DMA buffer: [22, 128, 864 | 108G+pad | 66ones+pad]
    pk = np.zeros((NPACK, 128, PKC), np.float32)
    pk[:, :, :IMC] = im_packed
    pk[:, :PPART, IMC:IMC + PPART] = gbd
    pk[:, :PPART, IMC + PPART:IMC + GO] = ones_p
    pk = np.ascontiguousarray(pk.astype(bf16))

    # -ln(caption word norms); padded words -> 0 (their ns column is 0,
    # so sim = 0 and each contributes exp(0)=1 to the row sum)
    cn = np.sqrt((s_z * s_z).sum(axis=2))                    # [64, 50]
    with np.errstate(divide="ignore"):
        nlc = np.where(cn > 0, -np.log(np.maximum(cn, 1e-30)), 0.0).astype(
            np.float32
        )
    padc = (W - s_l).astype(np.float32)                      # [64]

    in_maps = []
    for c in range(NCORES):
        cs = slice(CPC * c, CPC * (c + 1))
        s_cc = s_z[cs]                                        # [8, 50, 1024]
        sT = s_cc.transpose(2, 0, 1).reshape(D, NCW)          # [1024, 400]
        s_packed = np.ascontiguousarray(
            sT.reshape(KCH, 128, NCW).transpose(1, 0, 2).reshape(128, KCH * NCW)
            .astype(bf16)
        )
        nlc66 = np.broadcast_to(nlc[cs].reshape(1, NCW), (BP, NCW))
        padc66 = np.broadcast_to(padc[cs].reshape(1, CPC), (BP, CPC))
        in_maps.append(
            {
                "pk": pk,
                "s_packed": s_packed,
                "neglncn": np.ascontiguousarray(nlc66, dtype=np.float32),
                "padc66": np.ascontiguousarray(padc66, dtype=np.float32),
            }
        )
    return in_maps


def scores_from_results(res) -> np.ndarray:
    """res: list of per-core result dicts -> full [64, 64] score matrix."""
    rowc = np.concatenate([res[c]["scores8"] for c in range(NCORES)], axis=1)
    return np.log(np.maximum(rowc, 1e-30)) / LAM_LSE


def margin_loss(scores: np.ndarray) -> np.float32:
    scores = scores.astype(np.float32)
    diag = np.diag(scores).copy()
    cost_s = np.maximum(MARGIN + scores - diag[:, None], 0.0)
    cost_im = np.maximum(MARGIN + scores - diag[None, :], 0.0)
    np.fill_diagonal(cost_s, 0.0)
    np.fill_diagonal(cost_im, 0.0)
    return np.float32(cost_s.max(axis=1).sum() + cost_im.max(axis=0).sum())


def kernel(im: np.ndarray, s: np.ndarray, s_l: np.ndarray) -> np.ndarray:
    from concourse.bass_utils import run_bass_kernel_spmd

    if "nc" not in _PROGRAM_CACHE:
        _PROGRAM_CACHE["nc"] = build_program()
    nc = _PROGRAM_CACHE["nc"]

    in_maps = prepare_inputs(im, s, s_l)
    res = run_bass_kernel_spmd(nc, in_maps, list(range(NCORES))).results
    return margin_loss(scores_from_results(res))


# revision 3
# speedup vs baseline: 1.7400x; 1.3917x over previous
"""Trainium2 Bass kernel for nn_ContrastiveLoss (ragged_sequence), v3.

Math (see reference): a cross-attention t2i score matrix scores[i, c] over
B=64 images x B=64 captions, then a max-violation margin loss.

Sharding: captions are sharded 8-per-core across 8 NeuronCores; images are
replicated.  Each core computes its [64, 8] column block of the score
matrix; the tiny 64x64 margin-loss reduction runs on host.

v3 structure (fixes v2's HAM oscillation: PE idled during each group's
elementwise phase, re-throttling to 1.2 GHz):
  * software pipeline at pack granularity: sweep1(p) = DMA + A-matmuls +
    leaky/copy/square/word-norm; sweep2(p) = softmax weights + H + E*A/E*H
    + the two r-reduction matmuls.  sweep2 is emitted from a ready-queue,
    one pack per step, lagged behind sweep1 so the PE instruction queue
    always has a dense 8-matmul A burst between sweep2's small matmuls.
  * all matmuls bf16 (1 cycle/row; v1's f32r lowered to 4-pass fp32 HIGH)
  * the only in-loop ACT functions are Prelu/Copy/Exp (one table set; v1
    spent 59us thrashing ln/exp table loads)
  * rsqrt = magic-constant + 2 Newton steps on DVE, batched per group of
    8 packs; the 9x softmax temperature is folded into the last step
  * Z-cancellation: sim = ns / (cn*sqrt(ws)); no softmax-denominator
    accumulator, no reciprocals
  * engine balance per pack: PE 11 matmuls, ACT leaky+copy+exp,
    DVE reduce+newton+E*A+E*H, GPSIMD square+an
"""

import sys

if "/opt/trn_rl_repo" not in sys.path:
    sys.path.insert(0, "/opt/trn_rl_repo")

import numpy as np

B, R, W, D = 64, 36, 50, 1024
NCORES = 8
CPC = B // NCORES          # captions per core = 8
NCW = CPC * W              # 400 = per-core (c, w) columns
PACK = 3                   # images per pack
NPACK = 22                 # ceil(64 / 3) -> 66 rows incl. 2 pad images
BP = NPACK * PACK          # 66
PPART = PACK * R           # 108 partitions per pack
KCH = D // 128             # 8 contraction chunks
GO = PPART + BP            # 174 = G|ones combined columns
IMC = KCH * PPART          # 864 im columns per pack
PKC = IMC + GO             # 1038 combined pack columns

GROUPS = [list(range(0, 8)), list(range(8, 16)), list(range(16, 19)),
          list(range(19, 22))]
LAG_MIN = 2                # min packs between sweep1(p) and sweep2(p)

MARGIN = 0.2
LAM_SM = 9.0
LAM_LSE = 6.0

_PROGRAM_CACHE: dict = {}


def build_program(debug: bool = False):
    """Build the per-core Bass/Tile program (identical on all 8 cores)."""
    import concourse.bacc as bacc
    import concourse.mybir as mybir
    import concourse.tile as tile

    f32 = mybir.dt.float32
    bf16 = mybir.dt.bfloat16
    i32 = mybir.dt.int32
    AF = mybir.ActivationFunctionType
    ALU = mybir.AluOpType
    AX = mybir.AxisListType

    nc = bacc.Bacc("TRN2", target_bir_lowering=False, debug=debug)

    pk_d = nc.dram_tensor("pk", [NPACK, 128, PKC], bf16, kind="ExternalInput")
    s_d = nc.dram_tensor("s_packed", [128, KCH * NCW], bf16, kind="ExternalInput")
    nlc_d = nc.dram_tensor("neglncn", [BP, NCW], f32, kind="ExternalInput")
    pc_d = nc.dram_tensor("padc66", [BP, CPC], f32, kind="ExternalInput")
    out_d = nc.dram_tensor("scores8", [B, CPC], f32, kind="ExternalOutput")

    group_of = {}
    for g, packs in enumerate(GROUPS):
        for p in packs:
            group_of[p] = g

    MAXLIVE = max(len(g) for g in GROUPS) + LAG_MIN + 2  # al/a_sb/pk lifetime

    with tile.TileContext(nc) as tc:
        with (
            tc.tile_pool(name="const", bufs=1) as cpool,
            tc.tile_pool(name="pk", bufs=MAXLIVE) as pkpool,
            tc.tile_pool(name="ala", bufs=MAXLIVE) as alpool,
            tc.tile_pool(name="sqp", bufs=3) as sqpool,
            tc.tile_pool(name="anp", bufs=3) as anpool,
            tc.tile_pool(name="ep", bufs=4) as epool,
            tc.tile_pool(name="eaeh", bufs=3) as eapool,
            tc.tile_pool(name="nwt", bufs=2) as nwt,
            tc.tile_pool(name="ph2", bufs=2) as ph2,
            tc.tile_pool(name="psA", bufs=3, space="PSUM") as psA,
            tc.tile_pool(name="psH", bufs=2, space="PSUM") as psH,
            tc.tile_pool(name="psacc", bufs=1, space="PSUM") as psacc,
        ):
            s_sb = cpool.tile([128, KCH * NCW], bf16)
            nc.sync.dma_start(s_sb[:], s_d[:])
            nlc_sb = cpool.tile([BP, NCW], f32)
            nc.sync.dma_start(nlc_sb[:], nlc_d[:])
            pc_sb = cpool.tile([BP, CPC], f32)
            nc.sync.dma_start(pc_sb[:], pc_d[:])

            # magic constant for the rsqrt newton seed
            magic = cpool.tile([PPART, 1], i32)
            nc.vector.memset(magic[:], 0x5F3759DF)

            # persistent per-(b,r) x caption tiles for s2 / 9*rsqrt(s2)
            s2_all = cpool.tile([PPART, NPACK * CPC], f32)
            rn9_all = cpool.tile([PPART, NPACK * CPC], f32)

            # persistent PSUM accumulators (fp32): ns = sum_r E*A,
            # ws = sum_r E*H, accumulated over all 22 packs
            nz_acc = psacc.tile([BP, NCW], f32)
            wz_acc = psacc.tile([BP, NCW], f32)

            al_t: dict = {}
            asb_t: dict = {}
            pk_t: dict = {}

            def sweep1(p):
                pk_sb = pkpool.tile([128, PKC], bf16, tag="pk")
                nc.sync.dma_start(pk_sb[:], pk_d[p])
                pk_t[p] = pk_sb

                # A[108, 400] = sum_k im_pack_k^T @ s_k  (bf16)
                a_ps = psA.tile([PPART, NCW], f32)
                for k in range(KCH):
                    nc.tensor.matmul(
                        a_ps[:],
                        pk_sb[:, k * PPART:(k + 1) * PPART],
                        s_sb[:, k * NCW:(k + 1) * NCW],
                        start=(k == 0),
                        stop=(k == KCH - 1),
                    )

                # al = leaky_relu(A, 0.1) and a copy of raw A -> SBUF bf16
                al = alpool.tile([PPART, NCW], bf16, tag="al")
                nc.scalar.activation(al[:], a_ps[:], AF.Prelu, alpha=0.1)
                a_sb = alpool.tile([PPART, NCW], bf16, tag="asb")
                nc.scalar.activation(a_sb[:], a_ps[:], AF.Copy)
                al_t[p], asb_t[p] = al, a_sb

                # s2[108, 8] = sum_w al^2  (square on GPSIMD, reduce on DVE)
                sq = sqpool.tile([PPART, NCW], bf16, tag="sq")
                nc.gpsimd.tensor_mul(sq[:], al[:], al[:])
                nc.vector.tensor_reduce(
                    s2_all[:, p * CPC:(p + 1) * CPC],
                    sq[:].rearrange("p (c w) -> p c w", c=CPC),
                    AX.X,
                    ALU.add,
                )

            def newton(g):
                packs = GROUPS[g]
                lo, hi = packs[0] * CPC, (packs[-1] + 1) * CPC
                n = hi - lo
                x = nwt.tile([PPART, 8 * max(len(gg) for gg in GROUPS)], f32,
                             tag="x")
                x = x[:, :n]
                nc.vector.tensor_scalar_max(x, s2_all[:, lo:hi], 1e-30)
                t1 = nwt.tile([PPART, 8 * max(len(gg) for gg in GROUPS)], i32,
                              tag="t1")
                t1 = t1[:, :n]
                nc.vector.tensor_scalar(
                    t1, x.bitcast(i32), 1, None, op0=ALU.logical_shift_right
                )
                y0 = nwt.tile([PPART, 8 * max(len(gg) for gg in GROUPS)], f32,
                              tag="y0")
                y0 = y0[:, :n]
                nc.vector.tensor_tensor(
                    y0.bitcast(i32),
                    magic[:].broadcast_to([PPART, n]),
                    t1,
                    op=ALU.subtract,
                )
                # step 1: y1 = y0 * (1.5 - 0.5 * x * y0^2)
                a1 = nwt.tile([PPART, 8 * max(len(gg) for gg in GROUPS)], f32,
                              tag="a1")
                a1 = a1[:, :n]
                nc.vector.tensor_mul(a1, y0, y0)
                nc.vector.tensor_mul(a1, a1, x)
                nc.vector.tensor_scalar(a1, a1, -0.5, 1.5, op0=ALU.mult,
                                        op1=ALU.add)
                y1 = nwt.tile([PPART, 8 * max(len(gg) for gg in GROUPS)], f32,
                              tag="y1")
                y1 = y1[:, :n]
                nc.vector.tensor_mul(y1, a1, y0)
                # step 2 (folds in the 9x softmax temperature):
                # rn9 = y1 * (13.5 - 4.5 * x * y1^2) = 9 * rsqrt(x)
                b1 = nwt.tile([PPART, 8 * max(len(gg) for gg in GROUPS)], f32,
                              tag="b1")
                b1 = b1[:, :n]
                nc.vector.tensor_mul(b1, y1, y1)
                nc.vector.tensor_mul(b1, b1, x)
                nc.vector.tensor_scalar(b1, b1, -4.5, 13.5, op0=ALU.mult,
                                        op1=ALU.add)
                nc.vector.tensor_mul(rn9_all[:, lo:hi], b1, y1)

            def sweep2(p):
                first, last = (p == 0), (p == NPACK - 1)
                al, a_sb, pk_sb = al_t.pop(p), asb_t.pop(p), pk_t.pop(p)

                # an = al * (9 * rsqrt(s2)) broadcast over words  (GPSIMD)
                an = anpool.tile([PPART, NCW], f32, tag="an")
                nc.gpsimd.tensor_mul(
                    an[:].rearrange("p (c w) -> p c w", c=CPC),
                    al[:].rearrange("p (c w) -> p c w", c=CPC),
                    rn9_all[:, p * CPC:(p + 1) * CPC].broadcast_to(
                        [PPART, CPC, W]
                    ),
                )
                e = epool.tile([PPART, NCW], bf16, tag="e")
                nc.scalar.activation(e[:], an[:], AF.Exp)

                # H = Gbd @ E
                h_ps = psH.tile([PPART, NCW], f32)
                nc.tensor.matmul(
                    h_ps[:], pk_sb[:PPART, IMC:IMC + PPART], e[:],
                    start=True, stop=True,
                )

                # ea = E*A ; ns += ones^T ea
                ea = eapool.tile([PPART, NCW], bf16, tag="ea")
                nc.vector.tensor_mul(ea[:], e[:], a_sb[:])
                nc.tensor.matmul(
                    nz_acc[:], pk_sb[:PPART, IMC + PPART:IMC + GO], ea[:],
                    start=first, stop=last,
                )

                # eh = E*H ; ws += ones^T eh
                eh = eapool.tile([PPART, NCW], bf16, tag="eh")
                nc.vector.tensor_mul(eh[:], e[:], h_ps[:])
                nc.tensor.matmul(
                    wz_acc[:], pk_sb[:PPART, IMC + PPART:IMC + GO], eh[:],
                    start=first, stop=last,
                )

            # ---- software-pipelined emission ----
            ready: list = []       # packs whose newton is emitted
            emitted = 0            # sweep2 emitted count
            done_newton = set()
            for t in range(NPACK):
                sweep1(t)
                g = group_of[t]
                if t == GROUPS[g][-1]:
                    newton(g)
                    done_newton.add(g)
                    ready.extend(GROUPS[g])
                # drain one sweep2 per step, keeping a minimum lag
                if ready and ready[0] <= t - LAG_MIN:
                    sweep2(ready.pop(0))
            for p in ready:
                sweep2(p)

            # ---- phase 2: sim = ns/(cn*sqrt(ws)), LSE over words ----
            # split into halves so ACT/DVE stages of the two halves overlap
            rowz = ph2.tile([BP, CPC], f32, tag="rowz")
            for h in range(2):
                cs = slice(h * (NCW // 2), (h + 1) * (NCW // 2))
                ccs = slice(h * (CPC // 2), (h + 1) * (CPC // 2))
                wsm = ph2.tile([BP, NCW // 2], f32, tag="wsm")
                nc.vector.tensor_scalar_max(wsm[:], wz_acc[:, cs], 1e-30)
                tl = ph2.tile([BP, NCW // 2], f32, tag="tl")
                nc.scalar.activation(tl[:], wsm[:], AF.Ln)
                # u = -0.5*ln(ws) - ln(cn)
                u = ph2.tile([BP, NCW // 2], f32, tag="u")
                nc.vector.scalar_tensor_tensor(
                    u[:], tl[:], -0.5, nlc_sb[:, cs], ALU.mult, ALU.add
                )
                q = ph2.tile([BP, NCW // 2], f32, tag="q")
                nc.scalar.activation(q[:], u[:], AF.Exp)
                sim = ph2.tile([BP, NCW // 2], f32, tag="sim")
                nc.vector.tensor_mul(sim[:], q[:], nz_acc[:, cs])
                ee = ph2.tile([BP, NCW // 2], f32, tag="ee")
                nc.scalar.activation(ee[:], sim[:], AF.Exp, scale=LAM_LSE)
                nc.vector.tensor_reduce(
                    rowz[:, ccs],
                    ee[:].rearrange("p (c w) -> p c w", c=CPC // 2),
                    AX.X,
                    ALU.add,
                )
            rowc = ph2.tile([BP, CPC], f32, tag="rowc")
            nc.vector.tensor_sub(rowc[:], rowz[:], pc_sb[:])

            # host finishes with ln(rowc)/6
            nc.sync.dma_start(out_d[:], rowc[0:B, :])

    nc.compile()
    return nc


def prepare_inputs(im: np.ndarray, s: np.ndarray, s_l: np.ndarray):
    """Host-side input marshalling: shard captions, transpose to d-major,
    pack images into 3-image/108-partition packs, build the block-diagonal
    Gram + ones stationaries (fused with im into one per-pack DMA buffer),
    -ln(caption norms) and pad counts."""
    import ml_dtypes

    bf16 = ml_dtypes.bfloat16
    im = np.ascontiguousarray(np.asarray(im, np.float32))
    s = np.ascontiguousarray(np.asarray(s, np.float32))
    s_l = np.asarray(s_l).astype(np.int64)

    # zero out padded words so A columns for padded (c, w) are exactly 0
    wmask = (np.arange(W)[None, :] < s_l[:, None])          # [64, 50]
    s_z = s * wmask[:, :, None].astype(np.float32)

    # im packs: [22, 128, 864]
    imf = im.transpose(2, 0, 1).reshape(D, B * R)            # [1024, 2304]
    imf66 = np.zeros((D, BP * R), np.float32)
    imf66[:, : B * R] = imf
    im_packed = (
        imf66.reshape(KCH, 128, NPACK, PPART)
        .transpose(2, 1, 0, 3)
        .reshape(NPACK, 128, IMC)
    )

    # Gram matrices, block-diagonal per pack: [22, 108, 108]
    G = np.matmul(im, im.transpose(0, 2, 1))                 # [64, 36, 36] f32
    gbd = np.zeros((NPACK, PPART, PPART), np.float32)
    for j in range(PACK):
        for p in range(NPACK):
            b = PACK * p + j
            if b < B:
                gbd[p, R * j: R * (j + 1), R * j: R * (j + 1)] = G[b]

    # ones_p stationaries: [22, 108, 66], 3 block-ones columns at 3p
    ones_p = np.zeros((NPACK, PPART, BP), np.float32)
    for p in range(NPACK):
        for j in range(PACK):
            ones_p[p, R * j: R * (j + 1), PACK * p + j] = 1.0

    # fused per-pack DMA buffer: [22, 128, 864 | 108 G | 66 ones]
    pk = np.zeros((NPACK, 128, PKC), np.float32)
    pk[:, :, :IMC] = im_packed
    pk[:, :PPART, IMC:IMC + PPART] = gbd
    pk[:, :PPART, IMC + PPART:IMC + GO] = ones_p
    pk = np.ascontiguousarray(pk.astype(bf16))

    # -ln(caption word norms); padded words -> 0 (their ns column is 0,
    # so sim = 0 and each contributes exp(0)=1 to the row sum)
    cn = np.sqrt((s_z * s_z).sum(axis=2))                    # [64, 50]
    with np.errstate(divide="ignore"):
        nlc = np.where(cn > 0, -np.log(np.maximum(cn, 1e-30)), 0.0).astype(
            np.float32
        )
    padc = (W - s_l).astype(np.float32)                      # [64]

    in_maps = []
    for c in range(NCORES):
        cs = slice(CPC * c, CPC * (c + 1))
        s_cc = s_z[cs]                                        # [8, 50, 1024]
        sT = s_cc.transpose(2, 0, 1).reshape(D, NCW)          # [1024, 400]
        s_packed = np.ascontiguousarray(
            sT.reshape(KCH, 128, NCW).transpose(1, 0, 2).reshape(128, KCH * NCW)
            .astype(bf16)
        )
        nlc66 = np.broadcast_to(nlc[cs].reshape(1, NCW), (BP, NCW))
        padc66 = np.broadcast_to(padc[cs].reshape(1, CPC), (BP, CPC))
        in_maps.append(
            {
                "pk": pk,
                "s_packed": s_packed,
                "neglncn": np.ascontiguousarray(nlc66, dtype=np.float32),
                "padc66": np.ascontiguousarray(padc66, dtype=np.float32),
            }
        )
    return in_maps


def scores_from_results(res) -> np.ndarray:
    """res: list of per-core result dicts -> full [64, 64] score matrix."""
    rowc = np.concatenate([res[c]["scores8"] for c in range(NCORES)], axis=1)
    return np.log(np.maximum(rowc, 1e-30)) / LAM_LSE


def margin_loss(scores: np.ndarray) -> np.float32:
    scores = scores.astype(np.float32)
    diag = np.diag(scores).copy()
    cost_s = np.maximum(MARGIN + scores - diag[:, None], 0.0)
    cost_im = np.maximum(MARGIN + scores - diag[None, :], 0.0)
    np.fill_diagonal(cost_s, 0.0)
    np.fill_diagonal(cost_im, 0.0)
    return np.float32(cost_s.max(axis=1).sum() + cost_im.max(axis=0).sum())


def kernel(im: np.ndarray, s: np.ndarray, s_l: np.ndarray) -> np.ndarray:
    from concourse.bass_utils import run_bass_kernel_spmd

    if "nc" not in _PROGRAM_CACHE:
        _PROGRAM_CACHE["nc"] = build_program()
    nc = _PROGRAM_CACHE["nc"]

    in_maps = prepare_inputs(im, s, s_l)
    res = run_bass_kernel_spmd(nc, in_maps, list(range(NCORES))).results
    return margin_loss(scores_from_results(res))


# revision 5
# speedup vs baseline: 1.8714x; 1.0755x over previous
"""Trainium2 Bass kernel for nn_ContrastiveLoss (ragged_sequence), v4.

Math (see reference): a cross-attention t2i score matrix scores[i, c] over
B=64 images x B=64 captions, then a max-violation margin loss.

Sharding: captions are sharded 8-per-core across 8 NeuronCores; images are
replicated.  Each core computes its [64, 8] column block of the score
matrix; the tiny 64x64 margin-loss reduction runs on host.

v4 structure:
  * packs of 3 images (108 partitions) are processed in PAIRS: every
    elementwise instruction covers 800 columns (two packs), halving
    per-instruction overheads and semaphore traffic.  PSUM pair tiles
    place the two 400-column blocks at 512-element (bank) offsets; SBUF
    pair tiles are dense 800 columns.
  * software pipeline at pair granularity: sweep1 = DMA + 16 A-matmuls +
    Prelu/Copy/Square + word-norm reduce; sweep2 = an/E/H/E*A/E*H + the
    r-reduction matmuls, emitted from a ready queue lagged >= 2 steps
    behind the group's Newton batch so the PE queue always has a dense
    A-matmul burst covering the cross-engine chain latency (HAM stays at
    K=8/8).
  * all matmuls bf16; in-loop ACT functions only Prelu/Copy/Square/Exp
    (one table set); rsqrt = magic seed + 2 Newton steps on DVE with the
    9x temperature folded in; Z-cancellation (no softmax denominator).
  * engine split per pair: PE 22 matmuls, ACT Prelu+Copy+Square+Exp,
    DVE reduce+newton+E*A+E*H, GPSIMD an.
"""

import sys

if "/opt/trn_rl_repo" not in sys.path:
    sys.path.insert(0, "/opt/trn_rl_repo")

import numpy as np

B, R, W, D = 64, 36, 50, 1024
NCORES = 8
CPC = B // NCORES          # captions per core = 8
NCW = CPC * W              # 400 = per-core (c, w) columns
PACK = 3                   # images per pack
NPACK = 22                 # ceil(64 / 3) -> 66 rows incl. 2 pad images
NPAIR = NPACK // 2         # 11
BP = NPACK * PACK          # 66
PPART = PACK * R           # 108 partitions per pack
KCH = D // 128             # 8 contraction chunks
GO = PPART + BP            # 174 = G|ones combined columns
IMC = KCH * PPART          # 864 im columns per pack
PKC = IMC + GO             # 1038 combined pack columns
PB = 512                   # PSUM bank stride (fp32 elems) for pair tiles

# newton batches in pairs; smaller tail groups keep the drain short
PGROUPS = [[0, 1], [2, 3], [4, 5], [6, 7], [8], [9], [10]]
NEWTON_LAG = 2             # sweep2 waits >= this many steps after newton

MARGIN = 0.2
LAM_SM = 9.0
LAM_LSE = 6.0

_PROGRAM_CACHE: dict = {}


def build_program(debug: bool = False):
    """Build the per-core Bass/Tile program (identical on all 8 cores)."""
    import concourse.bacc as bacc
    import concourse.mybir as mybir
    import concourse.tile as tile

    f32 = mybir.dt.float32
    bf16 = mybir.dt.bfloat16
    i32 = mybir.dt.int32
    AF = mybir.ActivationFunctionType
    ALU = mybir.AluOpType
    AX = mybir.AxisListType

    nc = bacc.Bacc("TRN2", target_bir_lowering=False, debug=debug)

    pk_d = nc.dram_tensor("pk", [NPAIR, 128, 2 * PKC], bf16, kind="ExternalInput")
    s_d = nc.dram_tensor("s_packed", [128, KCH * NCW], bf16, kind="ExternalInput")
    nlc_d = nc.dram_tensor("neglncn", [BP, NCW], f32, kind="ExternalInput")
    pc_d = nc.dram_tensor("padc66", [BP, CPC], f32, kind="ExternalInput")
    out_d = nc.dram_tensor("scores8", [B, CPC], f32, kind="ExternalOutput")

    group_of = {}
    for g, pairs in enumerate(PGROUPS):
        for j in pairs:
            group_of[j] = g
    NMAX = 16 * max(len(gg) for gg in PGROUPS)

    MAXLIVE = 7  # al2/asb2/pk2 pair lifetime (sweep1 .. sweep2)

    with tile.TileContext(nc) as tc:
        with (
            tc.tile_pool(name="const", bufs=1) as cpool,
            tc.tile_pool(name="pk", bufs=MAXLIVE) as pkpool,
            tc.tile_pool(name="ala", bufs=MAXLIVE) as alpool,
            tc.tile_pool(name="sqp", bufs=2) as sqpool,
            tc.tile_pool(name="anp", bufs=2) as anpool,
            tc.tile_pool(name="ep", bufs=3) as epool,
            tc.tile_pool(name="eaeh", bufs=2) as eapool,
            tc.tile_pool(name="nwt", bufs=2) as nwt,
            tc.tile_pool(name="ph2", bufs=2) as ph2,
            tc.tile_pool(name="psA", bufs=2, space="PSUM") as psA,
            tc.tile_pool(name="psH", bufs=1, space="PSUM") as psH,
            tc.tile_pool(name="psacc", bufs=1, space="PSUM") as psacc,
        ):
            s_sb = cpool.tile([128, KCH * NCW], bf16)
            nc.sync.dma_start(s_sb[:], s_d[:])
            nlc_sb = cpool.tile([BP, NCW], f32)
            nc.sync.dma_start(nlc_sb[:], nlc_d[:])
            pc_sb = cpool.tile([BP, CPC], f32)
            nc.sync.dma_start(pc_sb[:], pc_d[:])

            # magic constant for the rsqrt newton seed
            magic = cpool.tile([PPART, 1], i32)
            nc.vector.memset(magic[:], 0x5F3759DF)

            # persistent per-(b,r) x caption tiles for s2 / 9*rsqrt(s2)
            s2_all = cpool.tile([PPART, NPACK * CPC], f32)
            rn9_all = cpool.tile([PPART, NPACK * CPC], f32)

            # persistent PSUM accumulators (fp32): ns = sum_r E*A,
            # ws = sum_r E*H, accumulated over all 22 packs
            nz_acc = psacc.tile([BP, NCW], f32)
            wz_acc = psacc.tile([BP, NCW], f32)

            al_t: dict = {}
            asb_t: dict = {}
            pk_t: dict = {}

            def pair_view(t):
                """[108, 2, 400] view of a [108, 2*PB] PSUM pair tile."""
                return t[:].rearrange("p (u x) -> p u x", u=2)[:, :, 0:NCW]

            def sweep1(j):
                pk_sb = pkpool.tile([128, 2 * PKC], bf16, tag="pk")
                nc.sync.dma_start(pk_sb[:], pk_d[j])
                pk_t[j] = pk_sb

                # A[108, 2x400] = sum_k im_pack_k^T @ s_k  (bf16)
                a_ps = psA.tile([PPART, 2 * PB], f32)
                for u in range(2):
                    for k in range(KCH):
                        nc.tensor.matmul(
                            a_ps[:, u * PB:u * PB + NCW],
                            pk_sb[:, u * PKC + k * PPART:u * PKC + (k + 1) * PPART],
                            s_sb[:, k * NCW:(k + 1) * NCW],
                            start=(k == 0),
                            stop=(k == KCH - 1),
                        )

                # al = leaky_relu(A, 0.1), a_sb = raw A -> dense SBUF bf16
                al = alpool.tile([PPART, 2 * NCW], bf16, tag="al")
                nc.scalar.activation(
                    al[:].rearrange("p (u x) -> p u x", u=2),
                    pair_view(a_ps), AF.Prelu, alpha=0.1,
                )
                a_sb = alpool.tile([PPART, 2 * NCW], bf16, tag="asb")
                nc.scalar.activation(
                    a_sb[:].rearrange("p (u x) -> p u x", u=2),
                    pair_view(a_ps), AF.Copy,
                )
                al_t[j], asb_t[j] = al, a_sb

                # s2[108, 16] = sum_w al^2  (square on ACT, reduce on DVE)
                sq = sqpool.tile([PPART, 2 * NCW], bf16, tag="sq")
                nc.scalar.activation(sq[:], al[:], AF.Square)
                nc.vector.tensor_reduce(
                    s2_all[:, j * 2 * CPC:(j + 1) * 2 * CPC],
                    sq[:].rearrange("p (c w) -> p c w", c=2 * CPC),
                    AX.X,
                    ALU.add,
                )

            def newton(g):
                pairs = PGROUPS[g]
                lo, hi = pairs[0] * 2 * CPC, (pairs[-1] + 1) * 2 * CPC
                n = hi - lo

                def tl(tag, dt=f32):
                    t = nwt.tile([PPART, NMAX], dt, tag=tag, name=f"nwt_{tag}")
                    return t[:, :n]

                x = tl("x")
                nc.vector.tensor_scalar_max(x, s2_all[:, lo:hi], 1e-30)
                t1 = tl("t1", i32)
                nc.vector.tensor_scalar(
                    t1, x.bitcast(i32), 1, None, op0=ALU.logical_shift_right
                )
                y0 = tl("y0")
                nc.vector.tensor_tensor(
                    y0.bitcast(i32),
                    magic[:].broadcast_to([PPART, n]),
                    t1,
                    op=ALU.subtract,
                )
                # step 1: y1 = y0 * (1.5 - 0.5 * x * y0^2)
                a1 = tl("a1")
                nc.vector.tensor_mul(a1, y0, y0)
                nc.vector.tensor_mul(a1, a1, x)
                nc.vector.tensor_scalar(a1, a1, -0.5, 1.5, op0=ALU.mult,
                                        op1=ALU.add)
                y1 = tl("y1")
                nc.vector.tensor_mul(y1, a1, y0)
                # step 2 (folds in the 9x softmax temperature):
                # rn9 = y1 * (13.5 - 4.5 * x * y1^2) = 9 * rsqrt(x)
                b1 = tl("b1")
                nc.vector.tensor_mul(b1, y1, y1)
                nc.vector.tensor_mul(b1, b1, x)
                nc.vector.tensor_scalar(b1, b1, -4.5, 13.5, op0=ALU.mult,
                                        op1=ALU.add)
                nc.vector.tensor_mul(rn9_all[:, lo:hi], b1, y1)

            def sweep2(j):
                al, a_sb, pk_sb = al_t.pop(j), asb_t.pop(j), pk_t.pop(j)

                # an = al * (9 * rsqrt(s2)) broadcast over words  (GPSIMD)
                an = anpool.tile([PPART, 2 * NCW], f32, tag="an")
                nc.gpsimd.tensor_mul(
                    an[:].rearrange("p (c w) -> p c w", c=2 * CPC),
                    al[:].rearrange("p (c w) -> p c w", c=2 * CPC),
                    rn9_all[:, j * 2 * CPC:(j + 1) * 2 * CPC].broadcast_to(
                        [PPART, 2 * CPC, W]
                    ),
                )
                e = epool.tile([PPART, 2 * NCW], bf16, tag="e")
                nc.scalar.activation(e[:], an[:], AF.Exp)

                # H = Gbd @ E  (per pack)
                h_ps = psH.tile([PPART, 2 * PB], f32)
                for u in range(2):
                    nc.tensor.matmul(
                        h_ps[:, u * PB:u * PB + NCW],
                        pk_sb[:PPART, u * PKC + IMC:u * PKC + IMC + PPART],
                        e[:, u * NCW:(u + 1) * NCW],
                        start=True, stop=True,
                    )

                # ea = E*A ; ns += ones^T ea  (per pack)
                ea = eapool.tile([PPART, 2 * NCW], bf16, tag="ea")
                nc.vector.tensor_mul(ea[:], e[:], a_sb[:])
                for u in range(2):
                    p = 2 * j + u
                    nc.tensor.matmul(
                        nz_acc[:],
                        pk_sb[:PPART, u * PKC + IMC + PPART:u * PKC + IMC + GO],
                        ea[:, u * NCW:(u + 1) * NCW],
                        start=(p == 0), stop=(p == NPACK - 1),
                    )

                # eh = E*H ; ws += ones^T eh  (per pack)
                eh = eapool.tile([PPART, 2 * NCW], bf16, tag="eh")
                nc.vector.tensor_mul(
                    eh[:].rearrange("p (u x) -> p u x", u=2),
                    e[:].rearrange("p (u x) -> p u x", u=2),
                    pair_view(h_ps),
                )
                for u in range(2):
                    p = 2 * j + u
                    nc.tensor.matmul(
                        wz_acc[:],
                        pk_sb[:PPART, u * PKC + IMC + PPART:u * PKC + IMC + GO],
                        eh[:, u * NCW:(u + 1) * NCW],
                        start=(p == 0), stop=(p == NPACK - 1),
                    )

            # ---- software-pipelined emission over pair steps ----
            newton_step: dict = {}
            ready: list = []
            for t in range(NPAIR):
                sweep1(t)
                g = group_of[t]
                if t == PGROUPS[g][-1]:
                    newton(g)
                    for j in PGROUPS[g]:
                        newton_step[j] = t
                    ready.extend(PGROUPS[g])
                if ready and t - newton_step[ready[0]] >= NEWTON_LAG:
                    sweep2(ready.pop(0))
            for j in ready:
                sweep2(j)

            # ---- phase 2: sim = ns/(cn*sqrt(ws)), LSE over words ----
            # split into halves so ACT/DVE stages of the two halves overlap
            rowz = ph2.tile([BP, CPC], f32, tag="rowz")
            for h in range(2):
                cs = slice(h * (NCW // 2), (h + 1) * (NCW // 2))
                ccs = slice(h * (CPC // 2), (h + 1) * (CPC // 2))
                wsm = ph2.tile([BP, NCW // 2], f32, tag="wsm")
                nc.vector.tensor_scalar_max(wsm[:], wz_acc[:, cs], 1e-30)
                tln = ph2.tile([BP, NCW // 2], f32, tag="tl")
                nc.scalar.activation(tln[:], wsm[:], AF.Ln)
                # u = -0.5*ln(ws) - ln(cn)
                uu = ph2.tile([BP, NCW // 2], f32, tag="u")
                nc.vector.scalar_tensor_tensor(
                    uu[:], tln[:], -0.5, nlc_sb[:, cs], ALU.mult, ALU.add
                )
                q = ph2.tile([BP, NCW // 2], f32, tag="q")
                nc.scalar.activation(q[:], uu[:], AF.Exp)
                sim = ph2.tile([BP, NCW // 2], f32, tag="sim")
                nc.vector.tensor_mul(sim[:], q[:], nz_acc[:, cs])
                ee = ph2.tile([BP, NCW // 2], f32, tag="ee")
                nc.scalar.activation(ee[:], sim[:], AF.Exp, scale=LAM_LSE)
                nc.vector.tensor_reduce(
                    rowz[:, ccs],
                    ee[:].rearrange("p (c w) -> p c w", c=CPC // 2),
                    AX.X,
                    ALU.add,
                )
            rowc = ph2.tile([BP, CPC], f32, tag="rowc")
            nc.vector.tensor_sub(rowc[:], rowz[:], pc_sb[:])

            # host finishes with ln(rowc)/6
            nc.sync.dma_start(out_d[:], rowc[0:B, :])

    nc.compile()
    return nc


def prepare_inputs(im: np.ndarray, s: np.ndarray, s_l: np.ndarray):
    """Host-side input marshalling: shard captions, transpose to d-major,
    pack images into 3-image/108-partition packs (paired, one DMA buffer
    per 2 packs), block-diagonal Gram + ones stationaries, -ln(caption
    norms) and pad counts."""
    import ml_dtypes

    bf16 = ml_dtypes.bfloat16
    im = np.ascontiguousarray(np.asarray(im, np.float32))
    s = np.ascontiguousarray(np.asarray(s, np.float32))
    s_l = np.asarray(s_l).astype(np.int64)

    # zero out padded words so A columns for padded (c, w) are exactly 0
    wmask = (np.arange(W)[None, :] < s_l[:, None])          # [64, 50]
    s_z = s * wmask[:, :, None].astype(np.float32)

    # im packs: [22, 128, 864]
    imf = im.transpose(2, 0, 1).reshape(D, B * R)            # [1024, 2304]
    imf66 = np.zeros((D, BP * R), np.float32)
    imf66[:, : B * R] = imf
    im_packed = (
        imf66.reshape(KCH, 128, NPACK, PPART)
        .transpose(2, 1, 0, 3)
        .reshape(NPACK, 128, IMC)
    )

    # Gram matrices, block-diagonal per pack: [22, 108, 108]
    G = np.matmul(im, im.transpose(0, 2, 1))                 # [64, 36, 36] f32
    gbd = np.zeros((NPACK, PPART, PPART), np.float32)
    for jj in range(PACK):
        for p in range(NPACK):
            b = PACK * p + jj
            if b < B:
                gbd[p, R * jj: R * (jj + 1), R * jj: R * (jj + 1)] = G[b]

    # ones_p stationaries: [22, 108, 66], 3 block-ones columns at 3p
    ones_p = np.zeros((NPACK, PPART, BP), np.float32)
    for p in range(NPACK):
        for jj in range(PACK):
            ones_p[p, R * jj: R * (jj + 1), PACK * p + jj] = 1.0

    # fused per-pack buffer [22, 128, 864 | 108 G | 66 ones], then paired
    pkb = np.zeros((NPACK, 128, PKC), np.float32)
    pkb[:, :, :IMC] = im_packed
    pkb[:, :PPART, IMC:IMC + PPART] = gbd
    pkb[:, :PPART, IMC + PPART:IMC + GO] = ones_p
    pk = np.ascontiguousarray(
        pkb.reshape(NPAIR, 2, 128, PKC).transpose(0, 2, 1, 3)
        .reshape(NPAIR, 128, 2 * PKC).astype(bf16)
    )

    # -ln(caption word norms); padded words -> 0 (their ns column is 0,
    # so sim = 0 and each contributes exp(0)=1 to the row sum)
    cn = np.sqrt((s_z * s_z).sum(axis=2))                    # [64, 50]
    with np.errstate(divide="ignore"):
        nlc = np.where(cn > 0, -np.log(np.maximum(cn, 1e-30)), 0.0).astype(
            np.float32
        )
    padc = (W - s_l).astype(np.float32)                      # [64]

    in_maps = []
    for c in range(NCORES):
        cs = slice(CPC * c, CPC * (c + 1))
        s_cc = s_z[cs]                                        # [8, 50, 1024]
        sT = s_cc.transpose(2, 0, 1).reshape(D, NCW)          # [1024, 400]
        s_packed = np.ascontiguousarray(
            sT.reshape(KCH, 128, NCW).transpose(1, 0, 2).reshape(128, KCH * NCW)
            .astype(bf16)
        )
        nlc66 = np.broadcast_to(nlc[cs].reshape(1, NCW), (BP, NCW))
        padc66 = np.broadcast_to(padc[cs].reshape(1, CPC), (BP, CPC))
        in_maps.append(
            {
                "pk": pk,
                "s_packed": s_packed,
                "neglncn": np.ascontiguousarray(nlc66, dtype=np.float32),
                "padc66": np.ascontiguousarray(padc66, dtype=np.float32),
            }
        )
    return in_maps


def scores_from_results(res) -> np.ndarray:
    """res: list of per-core result dicts -> full [64, 64] score matrix."""
    rowc = np.concatenate([res[c]["scores8"] for c in range(NCORES)], axis=1)
    return np.log(np.maximum(rowc, 1e-30)) / LAM_LSE


def margin_loss(scores: np.ndarray) -> np.float32:
    scores = scores.astype(np.float32)
    diag = np.diag(scores).copy()
    cost_s = np.maximum(MARGIN + scores - diag[:, None], 0.0)
    cost_im = np.maximum(MARGIN + scores - diag[None, :], 0.0)
    np.fill_diagonal(cost_s, 0.0)
    np.fill_diagonal(cost_im, 0.0)
    return np.float32(cost_s.max(axis=1).sum() + cost_im.max(axis=0).sum())


def kernel(im: np.ndarray, s: np.ndarray, s_l: np.ndarray) -> np.ndarray:
    from concourse.bass_utils import run_bass_kernel_spmd

    if "nc" not in _PROGRAM_CACHE:
        _PROGRAM_CACHE["nc"] = build_program()
    nc = _PROGRAM_CACHE["nc"]

    in_maps = prepare_inputs(im, s, s_l)
    res = run_bass_kernel_spmd(nc, in_maps, list(range(NCORES))).results
    return margin_loss(scores_from_results(res))
